# revision 10
# baseline (speedup 1.0000x reference)
"""Distributed causal multi-head attention for 8 Trainium2 NeuronCores.

Problem: B=2, S=2048, D=1024, H=16 heads (hd=64), fp32.
    qkv = x @ w_qkv + b_qkv ; causal softmax attention ; out = attn @ w_proj + b_proj

Distribution: core c -> (batch b = c//4, head group g = c%4 -> heads [4g, 4g+4)).
Transposed dataflow (channels on partitions, sequence on free axis); x arrives
host-transposed.  v2 of the kernel: same math as v1, restructured for engine
overlap:
  - The softmax exp stream on the Scalar engine (~75us) is the second-longest
    engine total after the PE (~100us); v1 serialized scores->exp->PV per tile
    so both engines idled.  v2 lags PV one chunk behind exp and interleaves
    ACT-independent matmul work (pair-1 QKV projection, V projection, output
    projection) between attention chunks so the PE never drains while ACT
    catches up.
  - Short causal tiles (N=256/128) are packed into one 512-col PSUM score
    slot and exp'd with a single activation call (76 calls instead of 116).
  - Group order p0:{0,2,1,3}, p1:{0,2} | A2A#1 | p1:{1,3} + proj(sq0) |
    A2A#2 | proj tail, so both collectives overlap compute.
  - Priority-ordered DMA: pair-0 qk weights, then x slab 0 in d-halves, so
    the first matmul starts ~5us in instead of ~15us.
  - Output shipped as bf16 (halves the out DMA); host casts back to f32.
"""

import os
import sys

sys.path.insert(0, "/opt/trn_rl_repo")

import numpy as np

import concourse.bass as bass
import concourse.tile as tile
from concourse import bacc, mybir
from concourse.bass_utils import run_bass_kernel_spmd

B, S, D = 2, 2048, 1024
H = 16
HD = 64
P = 128
N_CORES = 8
HPC = 4           # heads per core
DCH = D // P      # 8 contraction chunks
NQT = S // P      # 16 q tiles of 128
SCALE = 1.0 / 8.0  # 1/sqrt(hd)
NEG = -1.0e9

F32 = mybir.dt.float32
BF16 = mybir.dt.bfloat16


def attn_chunks(k):
    """Pack the kv-tile iterations of q-group k into <=512-col score chunks."""
    T = 13 + k
    chunks, cur, used = [], [], 0
    for t in range(T):
        s0 = max(0, (t - k + 3) // 4)
        N = (4 - s0) * P
        if used + N > 512:
            chunks.append(cur)
            cur, used = [], 0
        cur.append((t, used, N, s0))
        used += N
    if cur:
        chunks.append(cur)
    return chunks


def build():
    nc = bacc.Bacc(num_devices=N_CORES)

    xT = nc.declare_dram_parameter("xT", [D, S], BF16, isOutput=False)
    # columns reordered host-side: [q_p0 | k_p0 | q_p1 | k_p1] (128 each)
    w_qk = nc.declare_dram_parameter("w_qk", [D, 2 * HPC * HD], BF16, isOutput=False)
    w_v = nc.declare_dram_parameter("w_v", [D, HPC * HD], BF16, isOutput=False)
    consts = nc.declare_dram_parameter("consts", [P, 132], F32, isOutput=False)
    b_v = nc.declare_dram_parameter("b_v", [1, HPC * HD], BF16, isOutput=False)
    w_proj = nc.declare_dram_parameter("w_proj", [D, D], BF16, isOutput=False)
    b_proj = nc.declare_dram_parameter("b_proj", [1, D], BF16, isOutput=False)
    # each core outputs rows [256c, 256c+256) of BOTH batches (bf16)
    out_ext = nc.declare_dram_parameter("out", [2, S // 8, D], BF16, isOutput=True)

    groups = [list(range(N_CORES))]

    with tile.TileContext(nc) as tc:
        with (
            tc.tile_pool(name="weights", bufs=1) as wpool,
            tc.tile_pool(name="xslab", bufs=4) as xpool,
            tc.tile_pool(name="qkT", bufs=1) as qkpool,
            tc.tile_pool(name="big", bufs=1) as bigpool,
            tc.tile_pool(name="prob", bufs=4) as ppool,
            tc.tile_pool(name="small", bufs=3) as spool,
            tc.tile_pool(name="pj", bufs=2) as pjpool,
            tc.tile_pool(name="dram", bufs=1, space="DRAM") as dpool,
            tc.tile_pool(name="psS", bufs=2, space="PSUM") as psS,   # scores 2 banks/slot
            tc.tile_pool(name="psV", bufs=2, space="PSUM") as psV,   # pv accumulators
            tc.tile_pool(name="psQ", bufs=2, space="PSUM") as psQ,   # qkv/proj groups
        ):
            a2a_in = dpool.tile([N_CORES, HPC * HD, P], BF16, tag="a2a_in")
            a2a_out = dpool.tile([N_CORES, HPC * HD, P], BF16, tag="a2a_out")
            a2a_in2 = dpool.tile([N_CORES, HPC * HD, P], BF16, tag="a2a_in2")
            a2a_out2 = dpool.tile([N_CORES, HPC * HD, P], BF16, tag="a2a_out2")

            # ---- DMA priority order ----
            wqk_sb = wpool.tile([P, DCH, 2 * HPC * HD], BF16)
            nc.sync.dma_start(
                out=wqk_sb[:, :, 0:256],
                in_=w_qk[:, 0:256].rearrange("(o p) c -> p o c", p=P),
            )
            consts_sb = wpool.tile([P, 132], F32)
            nc.sync.dma_start(out=consts_sb[:], in_=consts[:, :])
            xsl_list = [
                xpool.tile([P, DCH, 512], BF16, tag="xslab", name=f"xsl{st}")
                for st in range(4)
            ]
            for dh in range(2):
                dsl = slice(dh * 4, dh * 4 + 4)
                nc.sync.dma_start(
                    out=xsl_list[0][:, dsl, :],
                    in_=xT[:, :].rearrange("(o p) s -> p o s", p=P)[:, dsl, 0:512],
                )
            for st in range(1, 4):
                nc.sync.dma_start(
                    out=xsl_list[st][:],
                    in_=xT[:, :].rearrange("(o p) s -> p o s", p=P)[:, :, st * 512:(st + 1) * 512],
                )
            wv_sb = wpool.tile([P, DCH, HPC * HD], BF16)
            nc.sync.dma_start(out=wv_sb[:], in_=w_v[:, :].rearrange("(o p) c -> p o c", p=P))
            bv_sb = wpool.tile([1, HPC * HD], BF16)
            nc.sync.dma_start(out=bv_sb[:], in_=b_v[:, :])
            nc.sync.dma_start(
                out=wqk_sb[:, :, 256:512],
                in_=w_qk[:, 256:512].rearrange("(o p) c -> p o c", p=P),
            )
            wproj_sb = wpool.tile([P, DCH, D], BF16)
            nc.sync.dma_start(out=wproj_sb[:], in_=w_proj[:, :].rearrange("(o p) c -> p o c", p=P))
            bproj_sb = wpool.tile([1, D], BF16)
            nc.sync.dma_start(out=bproj_sb[:], in_=b_proj[:, :])

            bqk_sb = consts_sb[:, 0:4]
            mask_sb = wpool.tile([P, P], F32)
            nc.vector.tensor_copy(out=mask_sb[:], in_=consts_sb[:, 4:132])
            ones_sb = wpool.tile([1, P], BF16)
            nc.vector.memset(ones_sb[:], 1.0)

            # qkT layout: [128, ct, 2048]; ct: 0=q_p0, 1=k_p0, 2=q_p1, 3=k_p1
            qkT_sb = qkpool.tile([P, 4, S], BF16)
            # V': [128 kv_inner, 16 kv_outer, 4*65]; col 65h+64 = 1.0 (softmax denom)
            v1_sb = bigpool.tile([P, NQT, HPC * 65], BF16)
            nc.gpsimd.memset(v1_sb[:], 1.0)
            # attn outT: [128, pair, 2048] bf16
            aT_sb = bigpool.tile([P, 2, S], BF16)
            out_sb = bigpool.tile([P, 2, 2, D], BF16)

            # ---- unit emitters (PE work chunks used as attention filler) ----
            def qk_unit(pair, st):
                """q,k projection for one pair, one s-slab (two 128-col tiles)."""
                xsl = xsl_list[st]
                for j in range(2):
                    ct = 2 * pair + j
                    ps = psQ.tile([P, 512], F32, tag="mm")
                    for d in range(DCH):
                        nc.tensor.matmul(
                            ps[:],
                            wqk_sb[:, d, ct * P:(ct + 1) * P],
                            xsl[:, d, :],
                            start=(d == 0),
                            stop=(d == DCH - 1),
                        )
                    nc.vector.tensor_scalar_add(
                        qkT_sb[:, ct, st * 512:(st + 1) * 512], ps[:], bqk_sb[:, ct:ct + 1]
                    )

            def v_unit(t16):
                """V projection for one 128-row s-tile (all 4 heads)."""
                st, sq = divmod(t16, 4)
                xsl = xsl_list[st]
                ps_full = psQ.tile([P, 512], F32, tag="mm", name="vacc")
                ps = ps_full[:, :HPC * HD]
                nc.tensor.matmul(ps[:], ones_sb[:, :], bv_sb[:, :], start=True, stop=False)
                for d in range(DCH):
                    nc.tensor.matmul(
                        ps[:],
                        xsl[:, d, sq * P:(sq + 1) * P],
                        wv_sb[:, d, :],
                        start=False,
                        stop=(d == DCH - 1),
                    )
                for h in range(HPC):
                    nc.vector.tensor_copy(
                        out=v1_sb[:, t16, h * 65:h * 65 + HD],
                        in_=ps[:, h * HD:(h + 1) * HD],
                    )

            pjT_tiles = {}

            def pjT_unit(sq, b2, a2a_o):
                pjT = pjpool.tile([P, DCH, P], BF16, tag="pjT", name=f"pjT{sq}{b2}")
                nc.sync.dma_start(
                    out=pjT[:],
                    in_=a2a_o[b2 * 4:(b2 + 1) * 4, :, :]
                    .rearrange("g (t pp) f -> pp (g t) f", pp=P),
                )
                pjT_tiles[(sq, b2)] = pjT

            def proj_unit(sq, b2, dc):
                pjT = pjT_tiles[(sq, b2)]
                ps = psQ.tile([P, 512], F32, tag="mm", name="pacc")
                nc.tensor.matmul(
                    ps[:], ones_sb[:, :], bproj_sb[:, dc * 512:(dc + 1) * 512],
                    start=True, stop=False,
                )
                for ch in range(DCH):
                    nc.tensor.matmul(
                        ps[:],
                        pjT[:, ch, :],
                        wproj_sb[:, ch, dc * 512:(dc + 1) * 512],
                        start=False,
                        stop=(ch == DCH - 1),
                    )
                nc.vector.tensor_copy(out=out_sb[:, b2, sq, dc * 512:(dc + 1) * 512], in_=ps[:])

            def out_unit(sq):
                nc.sync.dma_start(
                    out=out_ext[:, sq * P:(sq + 1) * P, :].rearrange("b pp d -> pp b d"),
                    in_=out_sb[:, :, sq, :],
                )

            # ---- attention group with chunk-packed exp, lag-1 PV, filler ----
            def attn_group(pair, k, filler, stride=1):
                T = 13 + k
                chunks = attn_chunks(k)
                qvA = qkT_sb[0:HD, 2 * pair, :].rearrange("p (i g) -> p i g", g=512)
                qvB = qkT_sb[HD:P, 2 * pair, :].rearrange("p (i g) -> p i g", g=512)
                kv_ct = 2 * pair + 1
                pvA = psV.tile([P, 512], F32, tag="pv")
                pvB = psV.tile([P, 512], F32, tag="pv")

                def emit_pv(pr, ch):
                    for (t, off, N, s0) in ch:
                        for hh, pv in ((0, pvA), (1, pvB)):
                            h = 2 * pair + hh
                            nc.tensor.matmul(
                                pv[0:65, s0 * P:512],
                                v1_sb[:, t, h * 65:(h + 1) * 65],
                                pr[:, hh, off:off + N],
                                start=(t == 0), stop=(t == T - 1),
                            )

                pending = None
                for ci, ch in enumerate(chunks):
                    used = ch[-1][1] + ch[-1][2]
                    sc_full = psS.tile([P, 2, 512], F32, tag="sc")
                    for (t, off, N, s0) in ch:
                        sc = sc_full[:, :, off:off + N]
                        nc.tensor.matmul(
                            sc[:, 0, :],
                            qkT_sb[0:HD, kv_ct, t * P:(t + 1) * P],
                            qvA[:, s0:4, k * P:(k + 1) * P],
                            start=True, stop=True, tile_position=(0, 0),
                        )
                        nc.tensor.matmul(
                            sc[:, 1, :],
                            qkT_sb[HD:P, kv_ct, t * P:(t + 1) * P],
                            qvB[:, s0:4, k * P:(k + 1) * P],
                            start=True, stop=True, tile_position=(64, 0),
                        )
                        if t >= k and (t - k) % 4 == 0:
                            nc.vector.tensor_add(
                                out=sc_full[:, :, off:off + P],
                                in0=sc_full[:, :, off:off + P],
                                in1=mask_sb[:, None, :].to_broadcast((P, 2, P)),
                            )
                    pr = ppool.tile([P, 2, 512], BF16, tag="prob")
                    nc.scalar.activation(
                        pr[:, :, 0:used], sc_full[:, :, 0:used],
                        mybir.ActivationFunctionType.Exp, scale=SCALE,
                    )
                    if filler and ci % stride == stride - 1:
                        filler.popleft()()
                    if pending is not None:
                        emit_pv(*pending)
                    pending = (pr, ch)
                emit_pv(*pending)

                # normalize + stage into aT
                for hh, pv in ((0, pvA), (1, pvB)):
                    h = 2 * pair + hh
                    base = hh * HD
                    sums_sb = spool.tile([1, 512], F32, tag="sums")
                    nc.vector.tensor_copy(out=sums_sb[:], in_=pv[64:65, :])
                    pvc = spool.tile([HD, 512], F32, tag="pvc")
                    nc.vector.tensor_copy(out=pvc[:], in_=pv[0:HD, :])
                    rec = spool.tile([1, 512], F32, tag="rec")
                    nc.vector.reciprocal_approx_fast(rec[:], sums_sb[:])
                    bc = spool.tile([HD, 512], F32, tag="bc")
                    nc.gpsimd.partition_broadcast(bc[:], rec[:])
                    nc.vector.tensor_tensor(
                        out=aT_sb[base:base + HD, pair, :]
                        .rearrange("p (i g) -> p i g", g=256)
                        [:, 4 * (k % 2):4 * (k % 2) + 4,
                         (k // 2) * P:(k // 2) * P + P],
                        in0=pvc[:].rearrange("p (i f) -> p i f", f=P),
                        in1=bc[:].rearrange("p (i f) -> p i f", f=P),
                        op=mybir.AluOpType.mult,
                    )

            def stage(a2a_buf, half):
                for t0 in range(2):
                    nc.sync.dma_start(
                        out=a2a_buf[:, t0 * P:(t0 + 1) * P, :]
                        .rearrange("s pp f -> pp s f"),
                        in_=aT_sb[:, t0, half * 1024:(half + 1) * 1024]
                        .rearrange("pp (s f) -> pp s f", f=P),
                    )

            from collections import deque

            # ---- emission schedule ----
            # E1: qk pair0 (attention p0 depends on all 4 slabs of ct0/ct1)
            for st in range(4):
                qk_unit(0, st)
            # E2: V tiles 0..3 (needed by the first PV steps)
            for t16 in range(4):
                v_unit(t16)

            # E3: p0 g0, filler = remaining V tiles (2 per chunk)
            vfill = deque()
            for t16 in range(4, 16, 2):
                def mk(a):
                    return lambda: (v_unit(a), v_unit(a + 1))
                vfill.append(mk(t16))
            attn_group(0, 0, vfill)
            # E4/E5: p0 g2, g1; filler = qk pair1 slabs (one ct per unit)
            qfill = deque()
            for st in range(4):
                for j in range(2):
                    def mkq(a, b):
                        return lambda: qk_unit_single(a, b)
                    qfill.append(mkq(st, j))

            def qk_unit_single(st, j):
                xsl = xsl_list[st]
                ct = 2 + j
                ps = psQ.tile([P, 512], F32, tag="mm")
                for d in range(DCH):
                    nc.tensor.matmul(
                        ps[:],
                        wqk_sb[:, d, ct * P:(ct + 1) * P],
                        xsl[:, d, :],
                        start=(d == 0),
                        stop=(d == DCH - 1),
                    )
                nc.vector.tensor_scalar_add(
                    qkT_sb[:, ct, st * 512:(st + 1) * 512], ps[:], bqk_sb[:, ct:ct + 1]
                )

            # E4: p0 g2 absorbs the pair-1 qk projection as filler
            attn_group(0, 2, qfill)
            while qfill:
                qfill.popleft()()
            # E5/E6: p1 g0, g2 -> first A2A fires at ~55% of attention
            attn_group(1, 0, deque())
            attn_group(1, 2, deque())
            stage(a2a_in, 0)
            nc.gpsimd.collective_compute(
                "AllToAll", mybir.AluOpType.bypass,
                ins=[a2a_in[:].opt()], outs=[a2a_out[:].opt()],
                replica_groups=groups,
            )
            # E7..E10: odd-parity groups; A2A#1 lands well before the pjT DMAs
            attn_group(0, 1, deque())
            pjfill = deque([
                lambda: pjT_unit(0, 0, a2a_out),
                lambda: pjT_unit(0, 1, a2a_out),
            ])
            attn_group(0, 3, pjfill, stride=4)
            attn_group(1, 1, deque())
            attn_group(1, 3, deque())
            stage(a2a_in2, 1)
            nc.gpsimd.collective_compute(
                "AllToAll", mybir.AluOpType.bypass,
                ins=[a2a_in2[:].opt()], outs=[a2a_out2[:].opt()],
                replica_groups=groups,
            )
            # all of proj(sq0) was held back to cover A2A#2 latency
            proj_unit(0, 0, 0)
            proj_unit(0, 0, 1)
            proj_unit(0, 1, 0)
            proj_unit(0, 1, 1)
            out_unit(0)
            pjT_unit(1, 0, a2a_out2)
            proj_unit(1, 0, 0)
            proj_unit(1, 0, 1)
            pjT_unit(1, 1, a2a_out2)
            proj_unit(1, 1, 0)
            proj_unit(1, 1, 1)
            out_unit(1)

    nc.compile()
    return nc


def make_in_maps(x, w_qkv, b_qkv, w_proj, b_proj):
    import ml_dtypes

    bf16 = ml_dtypes.bfloat16
    x = np.asarray(x, dtype=np.float32)
    w_qkv = np.asarray(w_qkv, dtype=np.float32)
    b_qkv = np.asarray(b_qkv, dtype=np.float32)
    w_proj_bf = np.ascontiguousarray(np.asarray(w_proj, dtype=np.float32).astype(bf16))
    b_proj_bf = np.ascontiguousarray(
        np.asarray(b_proj, dtype=np.float32).astype(bf16).reshape(1, -1)
    )

    # causal mask tile: mask[kv_local, q_local] = 0 if q >= kv else NEG
    m = np.where(np.arange(P)[None, :] >= np.arange(P)[:, None], 0.0, NEG).astype(np.float32)

    in_maps = []
    for c in range(N_CORES):
        b, g = divmod(c, 4)
        hs = slice(g * HPC * HD, (g + 1) * HPC * HD)
        xT = np.ascontiguousarray(x[b].T.astype(bf16))           # [D, S]
        w_q = w_qkv[:, 0:D][:, hs]
        w_k = w_qkv[:, D:2 * D][:, hs]
        # columns: [q_p0 | k_p0 | q_p1 | k_p1]
        w_qk = np.ascontiguousarray(np.concatenate(
            [w_q[:, 0:128], w_k[:, 0:128], w_q[:, 128:256], w_k[:, 128:256]], axis=1
        ).astype(bf16))
        w_v = np.ascontiguousarray(w_qkv[:, 2 * D:3 * D][:, hs].astype(bf16))
        bq = b_qkv[0:D][hs]
        bk = b_qkv[D:2 * D][hs]
        bqk = np.stack([bq[0:128], bk[0:128], bq[128:256], bk[128:256]], axis=1)  # [128, 4]
        cst = np.ascontiguousarray(np.concatenate([bqk, m], axis=1))     # [128, 132]
        bv = np.ascontiguousarray(b_qkv[2 * D:3 * D][hs].reshape(1, -1).astype(bf16))
        in_maps.append(
            {
                "xT": xT,
                "w_qk": w_qk,
                "w_v": w_v,
                "consts": cst,
                "b_v": bv,
                "w_proj": w_proj_bf,
                "b_proj": b_proj_bf,
            }
        )
    return in_maps


_NC_CACHE = None


def _install_ntff_shim():
    """Provide the antenv.axon_hooks module bass_utils wants for trace=True."""
    import sys as _sys
    import types

    if "antenv.axon_hooks" in _sys.modules:
        return
    try:
        from trn_agent_boot.trn_boot import _ntff_profile_via_ctypes

        hook = _ntff_profile_via_ctypes("/opt/axon/libaxon_pjrt.so")
    except Exception:
        hook = None
    mod = types.ModuleType("antenv.axon_hooks")
    mod._hook = hook
    mod.get_axon_ntff_profile_hook = lambda: mod._hook
    mod.set_axon_ntff_profile_hook = lambda h: setattr(mod, "_hook", h)
    _sys.modules["antenv.axon_hooks"] = mod


def kernel(x, w_qkv, b_qkv, w_proj, b_proj):
    global _NC_CACHE
    if _NC_CACHE is None:
        _NC_CACHE = build()
    nc = _NC_CACHE
    in_maps = make_in_maps(x, w_qkv, b_qkv, w_proj, b_proj)
    trace = bool(int(os.environ.get("KERNEL_TRACE", "0")))
    if trace:
        _install_ntff_shim()
    res = run_bass_kernel_spmd(
        nc,
        in_maps,
        core_ids=list(range(N_CORES)),
        trace=trace,
    )
    out = np.empty((B, S, D), dtype=np.float32)
    SB = S // 8
    for c in range(N_CORES):
        oc = res.results[c]["out"]  # [2, 256, 1024] bf16
        out[0, c * SB:(c + 1) * SB, :] = oc[0].astype(np.float32)
        out[1, c * SB:(c + 1) * SB, :] = oc[1].astype(np.float32)
    kernel.last_results = res
    return out
